# revision 7
# baseline (speedup 1.0000x reference)
"""Causal multi-head attention block on 8 TRN2 NeuronCores.

Sharding: tensor-parallel over heads (2 heads/core, both batches) for the
QKV projection + attention; an on-device AllToAll re-shards to
sequence-parallel for the output projection (Megatron-style).

v2 layout: the QKV projection and attention are merged into ONE streamed
loop over token chunks — attention chunk (b, j) is emitted immediately
after the projection of token chunk tc0 = 4*b + j (its last dependency),
so the scalar-engine exp stream starts ~3us into the kernel and overlaps
all remaining PE work. Operands are bf16 (halves DMA + makes narrow
diagonal-tile matmuls run at 1 cycle/row); PSUM accumulation stays f32.
Diagonal score tiles are narrowed to the causally-needed query range
(fully-masked query columns are never computed or exp'd).

Self-contained: hardcodes all shapes from the problem spec.
"""

import numpy as np
from contextlib import ExitStack

import concourse.bass as bass
import concourse.tile as tile
from concourse import bacc, mybir
from concourse.bass_utils import run_bass_kernel_spmd

F32R = mybir.dt.float32r
F32 = mybir.dt.float32
BF16 = mybir.dt.bfloat16
AF = mybir.ActivationFunctionType

B, T, C, H, HD = 2, 2048, 1024, 16, 64
NCORES = 8
BT = B * T            # 4096 global rows
TQ = 512              # q-chunk width
KT = 128              # k-tile height
NJ = T // TQ          # 4 q-chunks per batch (= per core)
NKK = T // KT         # 16 k-tiles per batch
NCT = C // 128        # 8 contraction tiles for projections
NTC = BT // TQ        # 8 global t-chunks
TSL = BT // NCORES    # 512 rows of final output per core
XT_SHAPE = [NCT, NTC, 128, TQ]  # tile-contiguous full x^T


def build(with_collective=True):
    nc = bacc.Bacc(None, target_bir_lowering=False)

    xt = nc.dram_tensor("xt", XT_SHAPE, BF16, kind="ExternalInput")
    wqkv = nc.dram_tensor("wqkv", [C, 3 * 128], BF16, kind="ExternalInput")
    bqkv = nc.dram_tensor("bqkv", [128, 3], F32, kind="ExternalInput")
    wout = nc.dram_tensor("wout", [C, C], BF16, kind="ExternalInput")
    bout = nc.dram_tensor("bout", [128, C], F32, kind="ExternalInput")
    out = nc.dram_tensor("out", [TSL, C], F32, kind="ExternalOutput")

    ident_d = nc.dram_tensor("ident", [128, 128], BF16, kind="ExternalInput")
    ones_d = nc.dram_tensor("ones", [128, 64], BF16, kind="ExternalInput")
    zeros_d = nc.dram_tensor("zeros", [64, TQ], BF16, kind="ExternalInput")
    a2a_in = nc.dram_tensor("a2a_in", [NCORES, 128, TQ], BF16)
    a2a_out = nc.dram_tensor("a2a_out", [NCORES, 128, TQ], BF16)

    with tile.TileContext(nc) as tc:
        _emit(nc, tc, xt, wqkv, bqkv, wout, bout, out, a2a_in, a2a_out,
              ident_d, ones_d, zeros_d, with_collective)
    nc.compile()
    return nc


def _emit(nc, tc, xt, wqkv, bqkv, wout, bout, out, a2a_in, a2a_out,
          ident_d, ones_d, zeros_d, with_collective, trunc=None):
    with ExitStack() as ctx:
        persist = ctx.enter_context(tc.tile_pool(name="persist", bufs=1))

        # persistent SBUF tensors, indexed by batch b (the core owns the
        # same 2 heads in both batches).
        qts = [persist.tile([128, T], BF16, tag=f"qt{p}", name=f"qt{p}")
               for p in range(2)]
        # zero-padded per-head K^T (head h lives in rows 64*(h%2);
        # the other 64 rows are zero so scores run as full K=128 matmuls)
        kts = [persist.tile([128, T], BF16, tag=f"kt{h}", name=f"kt{h}")
               for h in range(4)]
        va = persist.tile([128, 2, NKK, 192], BF16, tag="va")  # [V_e|ones|V_o]
        wsb = persist.tile([128, NCT, 384], BF16, tag="wsb")
        bsb = persist.tile([128, 3], F32, tag="bsb")
        wosb = persist.tile([128, NCT, C], BF16, tag="wo")
        bosb = persist.tile([128, C], F32, tag="bo")

        nc.sync.dma_start(wsb[:], wqkv[:].rearrange("(n p) c -> p n c", p=128))
        nc.sync.dma_start(bsb[:], bqkv[:])

        # zero padding of kts + VA ones (host constants)
        for h in range(4):
            dead = slice(64, 128) if h % 2 == 0 else slice(0, 64)
            for z in range(NJ):
                nc.sync.dma_start(kts[h][dead, TQ * z:TQ * (z + 1)], zeros_d[:])
        for p0 in range(2):
            for tt0 in range(NKK):
                nc.sync.dma_start(va[:, p0, tt0, 64:128], ones_d[:])

        # ---- merged projection + attention stream ----
        # Per token chunk tc0 = 4*b + j: the Q projection (+bias) runs
        # first, then the attention k-tile stream for chunk (b, j).  The
        # K/V projection matmul groups of the SAME chunk are emitted as
        # "units" interleaved between the non-diagonal k-tile steps, so
        # the PE chews projection work during the exp/mask wait of each
        # s->exp->av chain (diagonal tiles need this chunk's K/V, so all
        # units land before them).  V is transposed into va by the XBAR
        # DMA instead of PE identity-matmuls.
        with (
            tc.tile_pool(name="xtile", bufs=24) as xpool,
            tc.tile_pool(name="ps", bufs=2, space="PSUM") as pspool,
            tc.tile_pool(name="po", bufs=1, space="PSUM") as popool,
            tc.tile_pool(name="pj", bufs=1, space="PSUM") as pjpool,
            tc.tile_pool(name="vtile", bufs=3) as vpool,
            tc.tile_pool(name="ptp", bufs=6) as ptpool,
            tc.tile_pool(name="yt", bufs=3) as ytpool,
            tc.tile_pool(name="rt", bufs=3) as rtpool,
        ):
            for tc0 in range(NTC):
                b, j = tc0 // NJ, tc0 % NJ
                chunk = slice(TQ * j, TQ * (j + 1))

                xts = []
                for kc in range(NCT):
                    xtile = xpool.tile([128, TQ], BF16, tag="x",
                                       name=f"x{tc0}_{kc}")
                    nc.sync.dma_start(xtile[:], xt[kc, tc0])
                    xts.append(xtile)
                if tc0 == 1:
                    # wout prefetch: DMA has slack once the x stream thins
                    nc.sync.dma_start(wosb[:],
                                      wout[:].rearrange("(n p) c -> p n c",
                                                        p=128))
                    nc.sync.dma_start(bosb[:], bout[:])

                # Q projection upfront: every score matmul of this chunk
                # needs the chunk's own Q.
                ps01 = pjpool.tile([128, 2, TQ], F32, tag="pj",
                                   name=f"ps01_{tc0}")
                for kc in range(NCT):
                    nc.tensor.matmul(ps01[:, 0, :], wsb[:, kc, 0:128],
                                     xts[kc][:],
                                     start=(kc == 0), stop=(kc == NCT - 1))
                nc.vector.tensor_scalar_add(qts[b][:, chunk], ps01[:, 0, :],
                                            bsb[:, 0:1])

                # deferred K/V projection units
                def u_g1(half, b=b, tc0=tc0, ps01=ps01, chunk=chunk,
                         xts=xts):
                    for kc in range(4 * half, 4 * half + 4):
                        nc.tensor.matmul(ps01[:, 1, :],
                                         wsb[:, kc, 128:256], xts[kc][:],
                                         start=(kc == 0), stop=(kc == 7))
                    if half == 1:
                        nc.vector.tensor_scalar_add(
                            kts[2 * b][0:64, chunk], ps01[0:64, 1, :],
                            bsb[0:64, 1:2])
                        nc.vector.tensor_scalar_add(
                            kts[2 * b + 1][64:128, chunk],
                            ps01[64:128, 1, :], bsb[64:128, 1:2])

                vref = []

                def u_g2(half, b=b, j=j, tc0=tc0, xts=xts, vref=vref):
                    if half == 0:
                        vref.append(pjpool.tile([128, TQ], F32, tag="pj",
                                                name=f"ps2_{tc0}"))
                    ps2 = vref[0]
                    for kc in range(4 * half, 4 * half + 4):
                        nc.tensor.matmul(ps2[:], wsb[:, kc, 256:384],
                                         xts[kc][:],
                                         start=(kc == 0), stop=(kc == 7))
                    if half == 1:
                        vtile = vpool.tile([128, TQ], BF16, tag="v",
                                           name=f"v{tc0}")
                        nc.vector.tensor_scalar_add(vtile[:], ps2[:],
                                                    bsb[:, 2:3])
                        for q in range(4):
                            tt = j * 4 + q   # k-tile index in batch b
                            nc.sync.dma_start_transpose(
                                va[:, b, tt, 0:64],
                                vtile[0:64, 128 * q:128 * (q + 1)])
                            nc.sync.dma_start_transpose(
                                va[:, b, tt, 128:192],
                                vtile[64:128, 128 * q:128 * (q + 1)])

                units = [lambda: u_g1(0), lambda: u_g1(1),
                         lambda: u_g2(0), lambda: u_g2(1)]

                # -- attention chunk (b, j) --
                nkk = 4 * (j + 1)
                ndiag = 4 * j            # first diagonal k-tile index
                po = popool.tile([128, 2, TQ], F32, tag="po",
                                 name=f"po{b}_{j}")
                # unit i fires inside k-tile step floor(i*ndiag/4)
                unit_at = {}
                for i, u in enumerate(units):
                    s = min(i * max(ndiag, 1) // 4, max(ndiag - 1, 0))
                    unit_at.setdefault(s, []).append(u)
                if ndiag == 0:
                    for u in units:
                        u()
                for kk in range(nkk):
                    o = max(kk - ndiag, 0)   # diag suffix offset
                    lo = KT * o
                    ps_s = pspool.tile([128, 2, TQ], F32, tag="ps",
                                       name=f"s{b}_{j}_{kk}")
                    for h2 in range(2):
                        nc.tensor.matmul(
                            ps_s[:, h2, lo:],
                            kts[2 * b + h2][:, KT * kk:KT * (kk + 1)],
                            qts[b][:, TQ * j + lo:TQ * (j + 1)],
                            start=True, stop=True)
                    pt = ptpool.tile([128, 2, TQ], BF16, tag="pt",
                                     name=f"p{b}_{j}_{kk}")
                    nc.scalar.activation(pt[:, :, lo:], ps_s[:, :, lo:],
                                         AF.Exp)
                    if kk >= ndiag:
                        for h2 in range(2):
                            # aligned triangle: keep qf' >= r
                            nc.gpsimd.affine_select(
                                out=pt[:, h2, lo:],
                                in_=pt[:, h2, lo:],
                                compare_op=mybir.AluOpType.is_ge,
                                fill=0.0, base=0,
                                pattern=[[1, TQ - lo]],
                                channel_multiplier=-1)
                    elif kk in unit_at:
                        # projection work fills the PE's exp/mask wait
                        for u in unit_at.pop(kk):
                            u()
                    for h2 in range(2):
                        vs = slice(0, 128) if h2 == 0 else slice(64, 192)
                        nc.tensor.matmul(
                            po[:, h2, lo:],
                            va[:, b, kk, vs],
                            pt[:, h2, lo:],
                            start=(kk == 0), stop=(kk == nkk - 1))
                # normalize: h0 sums in rows 64:128, h1 sums in rows 0:64
                yt = ytpool.tile([128, TQ], BF16, tag="yt", name=f"y{b}_{j}")
                rt = rtpool.tile([128, TQ], F32, tag="rt", name=f"r{b}_{j}")
                nc.vector.reciprocal(rt[0:64, :], po[64:128, 0, :])
                nc.vector.tensor_mul(yt[0:64, :], po[0:64, 0, :], rt[0:64, :])
                nc.vector.reciprocal(rt[64:128, :], po[0:64, 1, :])
                nc.vector.tensor_mul(yt[64:128, :], po[64:128, 1, :],
                                     rt[64:128, :])
                nc.sync.dma_start(a2a_in[tc0, :, :], yt[:])

        if trunc == "attn":
            with tc.tile_pool(name="dumo2", bufs=1) as dpool2:
                d2 = dpool2.tile([128, TQ], F32, tag="d2")
                nc.vector.tensor_copy(d2[:], qts[0][0:128, 0:TQ])
                nc.sync.dma_start(out[0:128, 0:TQ], d2[:])
            return

        # ---- all-to-all (head-sharded -> t-sharded) ----
        if with_collective is True:
            nc.gpsimd.collective_compute(
                "AllToAll", mybir.AluOpType.bypass,
                replica_groups=[list(range(NCORES))],
                ins=[a2a_in[:]], outs=[a2a_out[:]])
        elif with_collective is False:
            nc.sync.dma_start(a2a_out[:], a2a_in[:])
        # else (None): timing mode — caller aliases a2a_out to a2a_in

        # ---- output projection (rows TSL per core) ----
        with (
            tc.tile_pool(name="yts", bufs=1) as ytspool,
            tc.tile_pool(name="osb", bufs=4) as osbpool,
            tc.tile_pool(name="pout", bufs=2, space="PSUM") as poutp,
        ):
            yts = ytspool.tile([128, NCT, TQ], BF16, tag="yts")
            for cc in range(NCT):
                nc.sync.dma_start(yts[:, cc, :], a2a_out[cc, :, :])

            for tt in range(TSL // 128):
                pos = poutp.tile([128, 2, TQ], F32, tag="po",
                                 name=f"pos{tt}")
                for cc in range(NCT):
                    for n in range(2):
                        nc.tensor.matmul(
                            pos[:, n, :], yts[:, cc, 128 * tt:128 * (tt + 1)],
                            wosb[:, cc, TQ * n:TQ * (n + 1)],
                            start=(cc == 0), stop=(cc == NCT - 1))
                for n in range(2):
                    osb = osbpool.tile([128, TQ], F32, tag="osb")
                    nc.vector.tensor_add(osb[:], pos[:, n, :],
                                         bosb[:, TQ * n:TQ * (n + 1)])
                    nc.sync.dma_start(
                        out[128 * tt:128 * (tt + 1), TQ * n:TQ * (n + 1)],
                        osb[:])


def make_core_inputs(x, w_qkv, b_qkv, w_out, b_out):
    """Host-side shard/transform. Returns list of per-core input dicts."""
    import ml_dtypes
    BF = np.dtype(ml_dtypes.bfloat16)
    x = np.asarray(x, np.float32)
    w_qkv = np.asarray(w_qkv, np.float32)
    b_qkv = np.asarray(b_qkv, np.float32)
    w_out = np.asarray(w_out, np.float32).astype(BF)
    b_out = np.asarray(b_out, np.float32)

    bout_rep = np.ascontiguousarray(
        np.broadcast_to(b_out, (128, C)).astype(np.float32))
    # tile-contiguous x^T: xt[kc, tc0, p, q] = x_flat[TQ*tc0+q, 128*kc+p]
    xt = np.ascontiguousarray(
        x.reshape(NTC, TQ, NCT, 128).transpose(2, 0, 3, 1)).astype(BF)
    in_maps = []
    for c in range(NCORES):
        s = slice(128 * c, 128 * (c + 1))
        wq = w_qkv[:, :C][:, s] * 0.125
        wk = w_qkv[:, C:2 * C][:, s]
        wv = w_qkv[:, 2 * C:][:, s]
        wc = np.ascontiguousarray(
            np.concatenate([wq, wk, wv], axis=1)).astype(BF)
        bc3 = np.ascontiguousarray(
            np.stack([b_qkv[:C][s] * 0.125, b_qkv[C:2 * C][s],
                      b_qkv[2 * C:][s]], axis=1).astype(np.float32))
        in_maps.append({
            "xt": xt, "wqkv": wc, "bqkv": bc3,
            "wout": w_out, "bout": bout_rep,
            "ident": np.eye(128, dtype=np.float32).astype(BF),
            "ones": np.ones((128, 64), BF),
            "zeros": np.zeros((64, TQ), BF),
        })
    return in_maps


_NC_CACHE = {}


def _make_cached_runner(nc):
    """Jit the SPMD executable once; subsequent calls only re-upload inputs."""
    import jax
    from jax.sharding import Mesh, PartitionSpec
    from jax.experimental.shard_map import shard_map
    from concourse.bass2jax import _bass_exec_p, install_neuronx_cc_hook

    install_neuronx_cc_hook()
    in_names, out_names, out_avals = [], [], []
    for alloc in nc.m.functions[0].allocations:
        if not isinstance(alloc, mybir.MemoryLocationSet):
            continue
        name = alloc.memorylocations[0].name
        if alloc.kind == "ExternalInput":
            in_names.append(name)
        elif alloc.kind == "ExternalOutput":
            out_names.append(name)
            out_avals.append(jax.core.ShapedArray(
                tuple(alloc.tensor_shape), mybir.dt.np(alloc.dtype)))
    n_params = len(in_names)
    all_in = list(in_names) + list(out_names)

    def _body(*args):
        outs = _bass_exec_p.bind(
            *args, out_avals=tuple(out_avals), in_names=tuple(all_in),
            out_names=tuple(out_names), lowering_input_output_aliases=(),
            sim_require_finite=True, sim_require_nnan=True, nc=nc)
        return tuple(outs)

    devices = jax.devices()[:NCORES]
    mesh = Mesh(np.asarray(devices), ("core",))
    spec = PartitionSpec("core")
    sharded = jax.jit(
        shard_map(_body, mesh=mesh,
                  in_specs=(spec,) * (n_params + len(out_names)),
                  out_specs=(spec,) * len(out_names), check_rep=False),
        keep_unused=True)
    zeros = [np.zeros((NCORES * a.shape[0], *a.shape[1:]), a.dtype)
             for a in out_avals]

    def run(in_maps):
        concat = [np.concatenate([np.asarray(m[nm]) for m in in_maps], axis=0)
                  for nm in in_names]
        outs = sharded(*concat, *zeros)
        return {nm: np.asarray(outs[i]) for i, nm in enumerate(out_names)}

    return run


def kernel(x, w_qkv, b_qkv, w_out, b_out):
    in_maps = make_core_inputs(x, w_qkv, b_qkv, w_out, b_out)
    if "nc" not in _NC_CACHE:
        _NC_CACHE["nc"] = build()
    nc = _NC_CACHE["nc"]
    try:
        if "run" not in _NC_CACHE:
            _NC_CACHE["run"] = _make_cached_runner(nc)
        outs = _NC_CACHE["run"](in_maps)
        full = outs["out"].reshape(NCORES * TSL, C)
    except Exception:
        res = run_bass_kernel_spmd(nc, in_maps, core_ids=list(range(NCORES)))
        full = np.concatenate([res.results[c]["out"] for c in range(NCORES)],
                              axis=0)
    return full.reshape(B, T, C)


# revision 8
# speedup vs baseline: 2.2456x; 2.2456x over previous
"""Causal multi-head attention block on 8 TRN2 NeuronCores.

Sharding: tensor-parallel over heads (2 heads/core, both batches) for the
QKV projection + attention; an on-device AllToAll re-shards to
sequence-parallel for the output projection (Megatron-style).

v2 layout: the QKV projection and attention are merged into ONE streamed
loop over token chunks — attention chunk (b, j) is emitted immediately
after the projection of token chunk tc0 = 4*b + j (its last dependency),
so the scalar-engine exp stream starts ~3us into the kernel and overlaps
all remaining PE work. Operands are bf16 (halves DMA + makes narrow
diagonal-tile matmuls run at 1 cycle/row); PSUM accumulation stays f32.
Diagonal score tiles are narrowed to the causally-needed query range
(fully-masked query columns are never computed or exp'd).

Self-contained: hardcodes all shapes from the problem spec.
"""

import numpy as np
from contextlib import ExitStack

import concourse.bass as bass
import concourse.tile as tile
from concourse import bacc, mybir
from concourse.bass_utils import run_bass_kernel_spmd

F32R = mybir.dt.float32r
F32 = mybir.dt.float32
BF16 = mybir.dt.bfloat16
AF = mybir.ActivationFunctionType

B, T, C, H, HD = 2, 2048, 1024, 16, 64
NCORES = 8
BT = B * T            # 4096 global rows
TQ = 512              # q-chunk width
KT = 128              # k-tile height
NJ = T // TQ          # 4 q-chunks per batch (= per core)
NKK = T // KT         # 16 k-tiles per batch
NCT = C // 128        # 8 contraction tiles for projections
NTC = BT // TQ        # 8 global t-chunks
TSL = BT // NCORES    # 512 rows of final output per core
XT_SHAPE = [NCT, NTC, 128, TQ]  # tile-contiguous full x^T


def build(with_collective=True):
    nc = bacc.Bacc(None, target_bir_lowering=False)

    xt = nc.dram_tensor("xt", XT_SHAPE, BF16, kind="ExternalInput")
    wqkv = nc.dram_tensor("wqkv", [C, 3 * 128], BF16, kind="ExternalInput")
    bqkv = nc.dram_tensor("bqkv", [128, 3], F32, kind="ExternalInput")
    wout = nc.dram_tensor("wout", [C, C], BF16, kind="ExternalInput")
    bout = nc.dram_tensor("bout", [128, C], F32, kind="ExternalInput")
    out = nc.dram_tensor("out", [TSL, C], F32, kind="ExternalOutput")

    ident_d = nc.dram_tensor("ident", [128, 128], BF16, kind="ExternalInput")
    ones_d = nc.dram_tensor("ones", [128, 64], BF16, kind="ExternalInput")
    zeros_d = nc.dram_tensor("zeros", [64, TQ], BF16, kind="ExternalInput")
    a2a_in = nc.dram_tensor("a2a_in", [NCORES, 128, TQ], BF16)
    a2a_out = nc.dram_tensor("a2a_out", [NCORES, 128, TQ], BF16)

    with tile.TileContext(nc) as tc:
        _emit(nc, tc, xt, wqkv, bqkv, wout, bout, out, a2a_in, a2a_out,
              ident_d, ones_d, zeros_d, with_collective)
    nc.compile()
    return nc


def _emit(nc, tc, xt, wqkv, bqkv, wout, bout, out, a2a_in, a2a_out,
          ident_d, ones_d, zeros_d, with_collective, trunc=None):
    with ExitStack() as ctx:
        persist = ctx.enter_context(tc.tile_pool(name="persist", bufs=1))

        # persistent SBUF tensors, indexed by batch b (the core owns the
        # same 2 heads in both batches).
        qts = [persist.tile([128, T], BF16, tag=f"qt{p}", name=f"qt{p}")
               for p in range(2)]
        # zero-padded per-head K^T (head h lives in rows 64*(h%2);
        # the other 64 rows are zero so scores run as full K=128 matmuls)
        kts = [persist.tile([128, T], BF16, tag=f"kt{h}", name=f"kt{h}")
               for h in range(4)]
        va = persist.tile([128, 2, NKK, 192], BF16, tag="va")  # [V_e|ones|V_o]
        wsb = persist.tile([128, NCT, 384], BF16, tag="wsb")
        bsb = persist.tile([128, 3], F32, tag="bsb")
        ident = persist.tile([128, 128], BF16, tag="ident")
        wosb = persist.tile([128, NCT, C], BF16, tag="wo")
        bosb = persist.tile([128, C], F32, tag="bo")

        nc.sync.dma_start(wsb[:], wqkv[:].rearrange("(n p) c -> p n c", p=128))
        nc.sync.dma_start(bsb[:], bqkv[:])

        # zero padding of kts + identity + VA ones (host constants)
        for h in range(4):
            dead = slice(64, 128) if h % 2 == 0 else slice(0, 64)
            for z in range(NJ):
                nc.sync.dma_start(kts[h][dead, TQ * z:TQ * (z + 1)], zeros_d[:])
        nc.sync.dma_start(ident[:], ident_d[:])
        for p0 in range(2):
            for tt0 in range(NKK):
                nc.sync.dma_start(va[:, p0, tt0, 64:128], ones_d[:])

        # ---- merged projection + attention stream ----
        with (
            tc.tile_pool(name="xtile", bufs=24) as xpool,
            tc.tile_pool(name="ps", bufs=2, space="PSUM") as pspool,
            tc.tile_pool(name="po", bufs=2, space="PSUM") as popool,
            tc.tile_pool(name="vtile", bufs=3) as vpool,
            tc.tile_pool(name="ptp", bufs=6) as ptpool,
            tc.tile_pool(name="yt", bufs=3) as ytpool,
            tc.tile_pool(name="rt", bufs=3) as rtpool,
        ):
            for tc0 in range(NTC):
                b, j = tc0 // NJ, tc0 % NJ
                chunk = slice(TQ * j, TQ * (j + 1))

                # -- projection of token chunk tc0 (Q,K,V for 2 heads) --
                xts = []
                for kc in range(NCT):
                    xtile = xpool.tile([128, TQ], BF16, tag="x",
                                       name=f"x{tc0}_{kc}")
                    nc.sync.dma_start(xtile[:], xt[kc, tc0])
                    xts.append(xtile)
                ps01 = pspool.tile([128, 2, TQ], F32, tag="ps",
                                   name=f"ps01_{tc0}")
                for g in range(2):
                    for kc in range(NCT):
                        nc.tensor.matmul(ps01[:, g, :],
                                         wsb[:, kc, 128 * g:128 * (g + 1)],
                                         xts[kc][:],
                                         start=(kc == 0), stop=(kc == NCT - 1))
                ps2 = pspool.tile([128, TQ], F32, tag="ps", name=f"ps2_{tc0}")
                for kc in range(NCT):
                    nc.tensor.matmul(ps2[:], wsb[:, kc, 256:384], xts[kc][:],
                                     start=(kc == 0), stop=(kc == NCT - 1))
                # biases; Q^T / K^T land in persistent per-batch tiles
                nc.vector.tensor_scalar_add(qts[b][:, chunk], ps01[:, 0, :],
                                            bsb[:, 0:1])
                nc.vector.tensor_scalar_add(kts[2 * b][0:64, chunk],
                                            ps01[0:64, 1, :], bsb[0:64, 1:2])
                nc.vector.tensor_scalar_add(kts[2 * b + 1][64:128, chunk],
                                            ps01[64:128, 1, :],
                                            bsb[64:128, 1:2])
                vtile = vpool.tile([128, TQ], BF16, tag="v", name=f"v{tc0}")
                nc.vector.tensor_scalar_add(vtile[:], ps2[:], bsb[:, 2:3])
                for q in range(4):
                    tt = j * 4 + q   # k-tile index in batch b
                    pst = pspool.tile([128, 128], BF16, tag="ps",
                                      name=f"pt{tc0}_{q}")
                    nc.tensor.matmul(pst[:], vtile[:, 128 * q:128 * (q + 1)],
                                     ident[:], is_transpose=True)
                    nc.vector.tensor_copy(va[:, b, tt, 0:64], pst[:, 0:64])
                    nc.vector.tensor_copy(va[:, b, tt, 128:192],
                                          pst[:, 64:128])

                if tc0 == 1:
                    # wout prefetch: DMA has slack once the x stream thins
                    nc.sync.dma_start(wosb[:],
                                      wout[:].rearrange("(n p) c -> p n c",
                                                        p=128))
                    nc.sync.dma_start(bosb[:], bout[:])

                # -- attention chunk (b, j): all deps now in SBUF --
                nkk = 4 * (j + 1)
                po = popool.tile([128, 2, TQ], F32, tag="po",
                                 name=f"po{b}_{j}")
                for kk in range(nkk):
                    o = max(kk - 4 * j, 0)   # diag suffix offset
                    lo = KT * o
                    ps_s = pspool.tile([128, 2, TQ], F32, tag="ps",
                                       name=f"s{b}_{j}_{kk}")
                    for h2 in range(2):
                        nc.tensor.matmul(
                            ps_s[:, h2, lo:],
                            kts[2 * b + h2][:, KT * kk:KT * (kk + 1)],
                            qts[b][:, TQ * j + lo:TQ * (j + 1)],
                            start=True, stop=True)
                    pt = ptpool.tile([128, 2, TQ], BF16, tag="pt",
                                     name=f"p{b}_{j}_{kk}")
                    nc.scalar.activation(pt[:, :, lo:], ps_s[:, :, lo:],
                                         AF.Exp)
                    if kk >= 4 * j:
                        for h2 in range(2):
                            # aligned triangle: keep qf' >= r
                            nc.gpsimd.affine_select(
                                out=pt[:, h2, lo:],
                                in_=pt[:, h2, lo:],
                                compare_op=mybir.AluOpType.is_ge,
                                fill=0.0, base=0,
                                pattern=[[1, TQ - lo]],
                                channel_multiplier=-1)
                    for h2 in range(2):
                        vs = slice(0, 128) if h2 == 0 else slice(64, 192)
                        nc.tensor.matmul(
                            po[:, h2, lo:],
                            va[:, b, kk, vs],
                            pt[:, h2, lo:],
                            start=(kk == 0), stop=(kk == nkk - 1))
                # normalize: h0 sums in rows 64:128, h1 sums in rows 0:64
                yt = ytpool.tile([128, TQ], BF16, tag="yt", name=f"y{b}_{j}")
                rt = rtpool.tile([128, TQ], F32, tag="rt", name=f"r{b}_{j}")
                nc.vector.reciprocal(rt[0:64, :], po[64:128, 0, :])
                nc.vector.tensor_mul(yt[0:64, :], po[0:64, 0, :], rt[0:64, :])
                nc.vector.reciprocal(rt[64:128, :], po[0:64, 1, :])
                nc.vector.tensor_mul(yt[64:128, :], po[64:128, 1, :],
                                     rt[64:128, :])
                nc.sync.dma_start(a2a_in[tc0, :, :], yt[:])

        if trunc == "attn":
            with tc.tile_pool(name="dumo2", bufs=1) as dpool2:
                d2 = dpool2.tile([128, TQ], F32, tag="d2")
                nc.vector.tensor_copy(d2[:], qts[0][0:128, 0:TQ])
                nc.sync.dma_start(out[0:128, 0:TQ], d2[:])
            return

        # ---- all-to-all (head-sharded -> t-sharded) ----
        if with_collective is True:
            nc.gpsimd.collective_compute(
                "AllToAll", mybir.AluOpType.bypass,
                replica_groups=[list(range(NCORES))],
                ins=[a2a_in[:]], outs=[a2a_out[:]])
        elif with_collective is False:
            nc.sync.dma_start(a2a_out[:], a2a_in[:])
        # else (None): timing mode — caller aliases a2a_out to a2a_in

        # ---- output projection (rows TSL per core) ----
        with (
            tc.tile_pool(name="yts", bufs=1) as ytspool,
            tc.tile_pool(name="osb", bufs=4) as osbpool,
            tc.tile_pool(name="pout", bufs=2, space="PSUM") as poutp,
        ):
            yts = ytspool.tile([128, NCT, TQ], BF16, tag="yts")
            for cc in range(NCT):
                nc.sync.dma_start(yts[:, cc, :], a2a_out[cc, :, :])

            for tt in range(TSL // 128):
                pos = poutp.tile([128, 2, TQ], F32, tag="po",
                                 name=f"pos{tt}")
                for cc in range(NCT):
                    for n in range(2):
                        nc.tensor.matmul(
                            pos[:, n, :], yts[:, cc, 128 * tt:128 * (tt + 1)],
                            wosb[:, cc, TQ * n:TQ * (n + 1)],
                            start=(cc == 0), stop=(cc == NCT - 1))
                for n in range(2):
                    osb = osbpool.tile([128, TQ], F32, tag="osb")
                    nc.vector.tensor_add(osb[:], pos[:, n, :],
                                         bosb[:, TQ * n:TQ * (n + 1)])
                    nc.sync.dma_start(
                        out[128 * tt:128 * (tt + 1), TQ * n:TQ * (n + 1)],
                        osb[:])


def make_core_inputs(x, w_qkv, b_qkv, w_out, b_out):
    """Host-side shard/transform. Returns list of per-core input dicts."""
    import ml_dtypes
    BF = np.dtype(ml_dtypes.bfloat16)
    x = np.asarray(x, np.float32)
    w_qkv = np.asarray(w_qkv, np.float32)
    b_qkv = np.asarray(b_qkv, np.float32)
    w_out = np.asarray(w_out, np.float32).astype(BF)
    b_out = np.asarray(b_out, np.float32)

    bout_rep = np.ascontiguousarray(
        np.broadcast_to(b_out, (128, C)).astype(np.float32))
    # tile-contiguous x^T: xt[kc, tc0, p, q] = x_flat[TQ*tc0+q, 128*kc+p]
    xt = np.ascontiguousarray(
        x.reshape(NTC, TQ, NCT, 128).transpose(2, 0, 3, 1)).astype(BF)
    in_maps = []
    for c in range(NCORES):
        s = slice(128 * c, 128 * (c + 1))
        wq = w_qkv[:, :C][:, s] * 0.125
        wk = w_qkv[:, C:2 * C][:, s]
        wv = w_qkv[:, 2 * C:][:, s]
        wc = np.ascontiguousarray(
            np.concatenate([wq, wk, wv], axis=1)).astype(BF)
        bc3 = np.ascontiguousarray(
            np.stack([b_qkv[:C][s] * 0.125, b_qkv[C:2 * C][s],
                      b_qkv[2 * C:][s]], axis=1).astype(np.float32))
        in_maps.append({
            "xt": xt, "wqkv": wc, "bqkv": bc3,
            "wout": w_out, "bout": bout_rep,
            "ident": np.eye(128, dtype=np.float32).astype(BF),
            "ones": np.ones((128, 64), BF),
            "zeros": np.zeros((64, TQ), BF),
        })
    return in_maps


_NC_CACHE = {}


def _make_cached_runner(nc):
    """Jit the SPMD executable once; subsequent calls only re-upload inputs."""
    import jax
    from jax.sharding import Mesh, PartitionSpec
    from jax.experimental.shard_map import shard_map
    from concourse.bass2jax import _bass_exec_p, install_neuronx_cc_hook

    install_neuronx_cc_hook()
    in_names, out_names, out_avals = [], [], []
    for alloc in nc.m.functions[0].allocations:
        if not isinstance(alloc, mybir.MemoryLocationSet):
            continue
        name = alloc.memorylocations[0].name
        if alloc.kind == "ExternalInput":
            in_names.append(name)
        elif alloc.kind == "ExternalOutput":
            out_names.append(name)
            out_avals.append(jax.core.ShapedArray(
                tuple(alloc.tensor_shape), mybir.dt.np(alloc.dtype)))
    n_params = len(in_names)
    all_in = list(in_names) + list(out_names)

    def _body(*args):
        outs = _bass_exec_p.bind(
            *args, out_avals=tuple(out_avals), in_names=tuple(all_in),
            out_names=tuple(out_names), lowering_input_output_aliases=(),
            sim_require_finite=True, sim_require_nnan=True, nc=nc)
        return tuple(outs)

    devices = jax.devices()[:NCORES]
    mesh = Mesh(np.asarray(devices), ("core",))
    spec = PartitionSpec("core")
    sharded = jax.jit(
        shard_map(_body, mesh=mesh,
                  in_specs=(spec,) * (n_params + len(out_names)),
                  out_specs=(spec,) * len(out_names), check_rep=False),
        keep_unused=True)
    zeros = [np.zeros((NCORES * a.shape[0], *a.shape[1:]), a.dtype)
             for a in out_avals]

    def run(in_maps):
        concat = [np.concatenate([np.asarray(m[nm]) for m in in_maps], axis=0)
                  for nm in in_names]
        outs = sharded(*concat, *zeros)
        return {nm: np.asarray(outs[i]) for i, nm in enumerate(out_names)}

    return run


def kernel(x, w_qkv, b_qkv, w_out, b_out):
    in_maps = make_core_inputs(x, w_qkv, b_qkv, w_out, b_out)
    if "nc" not in _NC_CACHE:
        _NC_CACHE["nc"] = build()
    nc = _NC_CACHE["nc"]
    try:
        if "run" not in _NC_CACHE:
            _NC_CACHE["run"] = _make_cached_runner(nc)
        outs = _NC_CACHE["run"](in_maps)
        full = outs["out"].reshape(NCORES * TSL, C)
    except Exception:
        res = run_bass_kernel_spmd(nc, in_maps, core_ids=list(range(NCORES)))
        full = np.concatenate([res.results[c]["out"] for c in range(NCORES)],
                              axis=0)
    return full.reshape(B, T, C)
